# revision 1
# baseline (speedup 1.0000x reference)
"""Trainium2 Bass kernel for DifferentiableMaxMedian (5x5 reflect-padded
max filter + soft-median filter, per-channel mix).

Contract: kernel(**inputs) takes FULL numpy inputs
  x: (4,32,256,256) f32, mix: (1,32,1,1) f32, beta_raw: () f32
and returns the full (4,32,256,256) f32 output.

Sharding: pure data parallel over B*C = 128 (b,c) slices -> 16 slices/core
across 8 cores.

Per-core layout: each slice is reflect-padded host-side to 260x260 and cut
into 32 row-strips of R=8 output rows. 4 slices x 32 strips = 128 SBUF
partitions per big tile (4 big tiles/core). A partition's free dim holds its
strip's 12 rows (8 + 4 halo) x 260 padded cols, fully contiguous in DRAM, so
each big tile loads with ONE DMA and every 5x5 window tap is a pure free-dim
AP offset.

Math: s_k = exp(C - beta*|v_k - mu|); the softmax ratio is invariant to C.
S = sum_k s_k and T = sum_k s_k*v_k accumulate on the TensorEngine as
identity-matmul PSUM accumulations (half-tiles of 4 rows so S/T double-buffer
within the 8 PSUM banks). out = lam*(T/S) + (1-lam)*max5x5.
"""

import numpy as np

from concourse import bass
import concourse.mybir as mybir
import concourse.tile_sem_assignment as _tsa
from concourse.ap import AP
from concourse.bass_utils import run_bass_kernel_spmd
from concourse.tile import TileContext
from concourse.tile_rust import add_dep_helper
from concourse.mybir import AluOpType as ALU
from concourse.mybir import ActivationFunctionType as AF

# All our DMAs issue from the sync engine's single HW-DGE ring (FIFO
# completion), so one bookkeeping sem lane is sound — and it keeps
# per-instruction wait counts under the per-engine ISA limits.
_tsa.NUM_HWDGE_SEMS = 1

F32 = mybir.dt.float32

B, C, H, W = 4, 32, 256, 256
NCORES = 8
SL = (B * C) // NCORES     # 16 slices per core
R = 8                      # output rows per strip
RP = R + 4                 # rows incl halo
WP = W + 4                 # padded width
NSTRIP = H // R            # 32 strips per slice
SPT = 128 // NSTRIP        # 4 slices per big tile
NT = SL // SPT             # 4 big tiles per core
HR = R // 2                # rows per half-tile (PSUM double-buffer unit)
# exp stability bias (cancels in the softmax ratio). Keeps S = sum_k
# exp(C - beta*|d_k|) well inside ACT-Ln's valid range (< 2^64): with the
# given inputs beta*min_k|d_k| <= ~10, so S is in ~[e^30, 25*e^40].
C_BIAS = 40.0


def _dram_ap(t, offset, dims):
    return AP(tensor=t[:].tensor, offset=offset, ap=dims)


def _elide_covered_waits(nc):
    """Drop sem waits already covered by an earlier wait on the same engine.

    Engines execute their instruction streams in order and Tile semaphores
    only increase, so a wait for (sem, value) when an earlier instruction on
    the same engine already waited for (sem, value' >= value) is a no-op.
    Tile's sem assignment emits these redundant waits at PSUM slot-reuse
    boundaries, where they overflow the LDWEIGHTS format's 1-wait budget.
    """
    skip = ("InstISA", "InstCustomDveAnt", "InstEventSemaphore")
    for b in nc.m.functions[0].blocks:
        seen = {}
        for ins in b.instructions:
            si = ins.sync_info
            if si is None or type(ins).__name__ in skip:
                continue
            w = si.on_wait
            if not w:
                continue
            eng = str(ins.engine).split(".")[-1]
            em = seen.setdefault(eng, {})
            kept = []
            for x in w:
                monotone = x.ant_name is not None and x.ant_name.startswith(
                    ("PE_", "DVE_", "Activation_", "Pool_", "SP_",
                     "DMAHW", "DMASW"))
                if (x.wait_mode != "sem-ge-imm" or x.wait_value is None
                        or x.wait_reg is not None or not monotone):
                    kept.append(x)
                    continue
                # ACT's same-engine self-waits are redundant (in-order
                # engine, writes cannot overtake) — but only drop them when
                # the instruction carries other waits too, to stay within
                # the 1-wait format budget without tripping CoreSim's
                # strict same-engine RAW detector on solo self-waits.
                if (eng == "Activation" and len(w) > 1
                        and x.ant_name.startswith(eng + "_")):
                    continue
                if em.get(x.ant_name, -1) >= x.wait_value:
                    continue
                kept.append(x)
                em[x.ant_name] = x.wait_value
            if len(kept) != len(w):
                si.on_wait = kept


def _split_excess_waits(nc):
    """Move all-but-one sync waits onto injected same-engine NoOps.

    TPB compute instruction formats encode a single embedded wait command
    (walrus rejects more with "Too many sync wait commands"). A NoOp on the
    same in-order engine carrying the extra waits immediately before the
    instruction is semantically identical: the engine stalls at the nop
    until the semaphores reach their thresholds, then proceeds.
    """
    n = 0
    skip = ("InstISA", "InstCustomDveAnt", "InstEventSemaphore")
    for b in nc.m.functions[0].blocks:
        out = []
        changed = False
        for ins in b.instructions:
            si = ins.sync_info
            w = list(si.on_wait) if si is not None and si.on_wait else []
            if len(w) > 1 and type(ins).__name__ not in skip:
                for x in w[:-1]:
                    n += 1
                    nop = mybir.InstNoOp(name=f"I-waitnop-{n}",
                                         engine=ins.engine)
                    nop.sync_info = mybir.SyncInfo(on_wait=[x], on_update=[])
                    try:
                        nop.debug = ins.debug
                    except Exception:
                        pass
                    out.append(nop)
                si.on_wait = [w[-1]]
                changed = True
            out.append(ins)
        if changed:
            b.instructions = out


def build_program(beta: float, split_waits: bool = True):
    nc = bass.Bass()
    xs = nc.declare_dram_parameter("xs", [SL, H + 4, W + 4], F32, isOutput=False)
    lam = nc.declare_dram_parameter("lam", [NT, 128, 1], F32, isOutput=False)
    ident = nc.declare_dram_parameter("ident", [128, 128], F32, isOutput=False)
    zeros = nc.declare_dram_parameter("zeros", [128, 512], F32, isOutput=False)
    y = nc.declare_dram_parameter("y", [SL, H, W], F32, isOutput=True)

    HW = H * W
    HPWP = (H + 4) * WP

    with TileContext(nc) as tc:
        with (
            tc.tile_pool(name="consts", bufs=1) as cpool,
            tc.tile_pool(name="main", bufs=2) as pool,
            tc.tile_pool(name="single", bufs=1) as spool,
            tc.psum_pool(name="ps", bufs=2) as pspool,
        ):
            itile = cpool.tile([128, 128], F32)
            nc.sync.dma_start(out=itile[:], in_=ident[:])
            ztile = cpool.tile([128, 512], F32)
            nc.sync.dma_start(out=ztile[:], in_=zeros[:])
            cbias = cpool.tile([128, 1], F32)
            nc.vector.memset(cbias[:], C_BIAS)

            psum_releases = {}
            for t in range(NT):
                # ---- load big tile t (4 slices x 32 strips) in one DMA ----
                X = pool.tile([128, RP, WP], F32, tag="X")
                nc.sync.dma_start(
                    out=X[:],
                    in_=_dram_ap(xs, t * SPT * HPWP,
                                 [[HPWP, SPT], [R * WP, NSTRIP], [1, RP * WP]]),
                )
                lamt = pool.tile([128, 1], F32, tag="lam")
                nc.sync.dma_start(out=lamt[:], in_=lam[t])

                # ---- separable 25-sum (for the mean) on DVE ----
                V = spool.tile([128, R, WP], F32, tag="V")
                nc.vector.tensor_tensor(V[:], X[:, 0:R, :], X[:, 1:R + 1, :], ALU.add)
                for dy in (2, 3, 4):
                    nc.vector.tensor_tensor(V[:], V[:], X[:, dy:dy + R, :], ALU.add)
                M = spool.tile([128, R, W], F32, tag="M")
                nc.vector.tensor_tensor(M[:], V[:, :, 0:W], V[:, :, 1:W + 1], ALU.add)
                for dx in (2, 3, 4):
                    nc.vector.tensor_tensor(M[:], M[:], V[:, :, dx:dx + W], ALU.add)

                # ---- separable 5x5 max (DVE) ----
                MV = spool.tile([128, R, WP], F32, tag="MV")
                nc.vector.tensor_tensor(MV[:], X[:, 0:R, :], X[:, 1:R + 1, :], ALU.max)
                for dy in (2, 3, 4):
                    nc.vector.tensor_tensor(MV[:], MV[:], X[:, dy:dy + R, :], ALU.max)
                MX = spool.tile([128, R, W], F32, tag="MX")
                nc.vector.tensor_tensor(MX[:], MV[:, :, 0:W], MV[:, :, 1:W + 1], ALU.max)
                for dx in (2, 3, 4):
                    nc.vector.tensor_tensor(MX[:], MX[:], MV[:, :, dx:dx + W], ALU.max)

                # ---- 25 taps: softmax-weighted sums on DVE/ACT/PE ----
                # Processed in two half-tiles (rows 0-3 / 4-7 per strip) so
                # the S/T PSUM accumulators take 2 banks each and
                # double-buffer (2 tags x 2 bufs x 2 banks = 8 banks). Each
                # accumulation group is opened by a zeros-rhs "warmup"
                # matmul that alone carries the PSUM slot-reuse drain wait
                # (the LDWEIGHTS format allows one sync wait only).
                for h in range(2):
                    r0 = h * HR
                    g = 2 * t + h
                    S_ps = pspool.tile([128, HR, W], F32, tag="S")
                    T_ps = pspool.tile([128, HR, W], F32, tag="T")
                    # A PE nop (CTRL format: multi-wait budget) syncs on the
                    # recycled slot's DVE readers so the warmup matmuls keep
                    # only their single PE-drain wait.
                    if g >= 2:
                        pe_nop = nc.tensor.nop()
                        for rel in psum_releases[g - 2]:
                            add_dep_helper(pe_nop.ins, rel.ins, sync=True,
                                           reason="psum slot reader sync")
                    else:
                        pe_nop = None
                    for cch in range(2):
                        sel = (slice(None), slice(2 * cch, 2 * cch + 2),
                               slice(None))
                        wm_t = nc.tensor.matmul(T_ps[sel], itile[:], ztile[:],
                                                start=True, stop=False)
                        wm_s = nc.tensor.matmul(S_ps[sel], itile[:], ztile[:],
                                                start=True, stop=False)
                        if pe_nop is not None:
                            add_dep_helper(wm_t.ins, pe_nop.ins, sync=True,
                                           reason="nop before warmup")
                            add_dep_helper(wm_s.ins, pe_nop.ins, sync=True,
                                           reason="nop before warmup")
                    ntap = 0
                    for dy in range(5):
                        for dx in range(5):
                            v_ap = X[:, r0 + dy:r0 + dy + HR, dx:dx + W]
                            d = pool.tile([128, HR, W], F32, tag="d")
                            # d = M*(-1/25) + v_k
                            nc.vector.scalar_tensor_tensor(
                                d[:], M[:, r0:r0 + HR, :], -1.0 / 25.0, v_ap,
                                ALU.mult, ALU.add)
                            # a = |d| in place: clear the f32 sign bit via
                            # an int32 bitcast (valid TS op, 2x mode)
                            di = d[:].bitcast(mybir.dt.int32)
                            nc.vector.tensor_scalar(di, di, 0x7FFFFFFF, None,
                                                    ALU.bitwise_and)
                            st = pool.tile([128, HR, W], F32, tag="st")
                            nc.scalar.activation(st[:], d[:], AF.Exp,
                                                 bias=cbias[:], scale=-beta)
                            pt = pool.tile([128, HR, W], F32, tag="pt")
                            nc.vector.tensor_tensor(pt[:], st[:], v_ap,
                                                    ALU.mult)
                            last = ntap == 24
                            for cch in range(2):
                                sel = (slice(None),
                                       slice(2 * cch, 2 * cch + 2),
                                       slice(None))
                                nc.tensor.matmul(T_ps[sel], itile[:], pt[sel],
                                                 start=False, stop=last)
                                nc.tensor.matmul(S_ps[sel], itile[:], st[sel],
                                                 start=False, stop=last)
                            ntap += 1

                    # ---- combine: out = lam*(T/S) + (1-lam)*max ----
                    # 1/S = exp(-ln S) on ACT (standard opcodes; Ln and Exp
                    # share the natural_log_exp table set). S spans
                    # ~[1e27, 3e31] here so both stay in range.
                    MXh = MX[:, r0:r0 + HR, :]
                    lnS = spool.tile([128, HR, W], F32, tag="lnS")
                    rel_s = nc.scalar.activation(lnS[:], S_ps[:], AF.Ln)
                    rS = spool.tile([128, HR, W], F32, tag="rS")
                    nc.scalar.activation(rS[:], lnS[:], AF.Exp, scale=-1.0)
                    med = spool.tile([128, HR, W], F32, tag="med")
                    rel_t = nc.vector.tensor_tensor(med[:], rS[:], T_ps[:],
                                                    ALU.mult)
                    psum_releases[g] = (rel_s, rel_t)
                    nc.vector.tensor_tensor(med[:], med[:], MXh, ALU.subtract)
                    out_t = pool.tile([128, HR, W], F32, tag="out")
                    nc.vector.scalar_tensor_tensor(
                        out_t[:], med[:], lamt[:], MXh, ALU.mult, ALU.add)

                    # ---- store half-tile ----
                    nc.sync.dma_start(
                        out=_dram_ap(y, t * SPT * HW + h * HR * W,
                                     [[HW, SPT], [R * W, NSTRIP],
                                      [1, HR * W]]),
                        in_=out_t[:],
                    )
    _elide_covered_waits(nc)
    if split_waits:
        # Mechanical transform for walrus's 1-wait instruction formats;
        # skip under CoreSim (its race detector requires sem updates on
        # every instruction, which the injected bare NoOps lack).
        _split_excess_waits(nc)
    return nc


def _make_inputs(x, mix, beta_raw):
    """Host-side sharding. Returns (beta, in_maps)."""
    x = np.ascontiguousarray(x, dtype=np.float32)
    mix = np.asarray(mix, dtype=np.float32).reshape(C)
    beta_raw = float(np.asarray(beta_raw, dtype=np.float32))
    beta = float(5.0 + 45.0 / (1.0 + np.exp(-beta_raw)))
    lam_c = (1.0 / (1.0 + np.exp(-mix.astype(np.float64)))).astype(np.float32)

    xs_all = np.pad(x.reshape(B * C, H, W), ((0, 0), (2, 2), (2, 2)),
                    mode="reflect")
    ident = np.eye(128, dtype=np.float32)
    zeros = np.zeros((128, 512), dtype=np.float32)
    in_maps = []
    for core in range(NCORES):
        sl0 = core * SL
        shard = np.ascontiguousarray(xs_all[sl0:sl0 + SL])
        lam_t = np.empty((NT, 128, 1), dtype=np.float32)
        for t in range(NT):
            for p in range(128):
                g_slice = sl0 + t * SPT + p // NSTRIP
                lam_t[t, p, 0] = lam_c[g_slice % C]
        in_maps.append({"xs": shard, "lam": lam_t, "ident": ident,
                        "zeros": zeros})
    return beta, in_maps


def kernel(x, mix, beta_raw):
    beta, in_maps = _make_inputs(x, mix, beta_raw)
    nc = build_program(beta)
    res = run_bass_kernel_spmd(nc, in_maps, list(range(NCORES))).results
    out = np.concatenate([res[i]["y"].reshape(SL, H, W) for i in range(NCORES)],
                         axis=0)
    return np.ascontiguousarray(out.reshape(B, C, H, W))



# revision 7
# speedup vs baseline: 1.7159x; 1.7159x over previous
"""Trainium2 Bass kernel for DifferentiableMaxMedian (5x5 reflect-padded
max filter + soft-median filter, per-channel mix).

Contract: kernel(**inputs) takes FULL numpy inputs
  x: (4,32,256,256) f32, mix: (1,32,1,1) f32, beta_raw: () f32
and returns the full (4,32,256,256) f32 output.

Sharding: pure data parallel over B*C = 128 (b,c) slices -> 16 slices/core
across 8 cores.

Per-core layout: each slice is reflect-padded host-side to 260x262 in BF16
and cut into 32 row-strips of R=8 output rows. 4 slices x 32 strips = 128
SBUF partitions per big tile (4 big tiles/core). A partition's free dim
holds its strip's 12 rows (8 + 4 halo) x 262 padded cols, contiguous in
DRAM, so each big tile loads with ONE DMA per column-parity copy and every
5x5 window tap is a pure free-dim AP offset. Two copies (even / odd column
base) keep every 16-bit DVE operand 4-byte aligned, which is what unlocks
the 2x_1P DVE mode for the hot per-tap tensor_tensor ops.

Math per tap k: s_k = exp(C - beta*|v_k - mu|); the softmax ratio is
invariant to C. The 25-tap window mean 25*mu is accumulated on the
TensorEngine as identity-matmul PSUM accumulations and evacuated through
the Scalar engine with a free -1/25 affine (nmu = -mu). S = sum_k s_k and
T = sum_k s_k*v_k accumulate on the TensorEngine in BF16 (PSUM keeps f32).
The combine runs in f32: out = lam*(T/S) + (1-lam)*max5x5, with
1/S = exp(-ln S) on ACT.
"""

import numpy as np
import ml_dtypes

from concourse import bass
import concourse.mybir as mybir
import concourse.tile_sem_assignment as _tsa
from concourse.ap import AP
from concourse.bass_utils import run_bass_kernel_spmd
from concourse.tile import TileContext
from concourse.mybir import AluOpType as ALU
from concourse.mybir import ActivationFunctionType as AF

# All our DMAs issue from the sync engine's single HW-DGE ring (FIFO
# completion), so one bookkeeping sem lane is sound — and it keeps
# per-instruction wait counts under the per-engine ISA limits.
_tsa.NUM_HWDGE_SEMS = 1

F32 = mybir.dt.float32
BF16 = mybir.dt.bfloat16

B, C, H, W = 4, 32, 256, 256
NCORES = 8
SL = (B * C) // NCORES     # 16 slices per core
R = 8                      # output rows per strip
RP = R + 4                 # rows incl halo
HP = H + 4                 # padded rows per slice
WP = W + 6                 # padded width (2 left, 4 right: odd-shift slack)
NSTRIP = H // R            # 32 strips per slice
SPT = 128 // NSTRIP        # 4 slices per big tile
NT = SL // SPT             # 4 big tiles per core
# exp stability bias (cancels in the softmax ratio). Keeps S = sum_k
# exp(C - beta*|d_k|) below the ACT-Ln valid range (< 2^64) while making
# S-underflow impossible for any window realizable from N(0,1) data.
C_BIAS = 40.0

SLICE_E = HP * WP          # elements per padded slice
STRIP_E = R * WP           # strip stride in elements
INNER_E = RP * WP          # elements per partition load
HW = H * W


def _dram_ap(t, offset, dims):
    return AP(tensor=t[:].tensor, offset=offset, ap=dims)


def _elide_covered_waits(nc):
    """Drop sem waits already covered by an earlier wait on the same engine.

    Engines execute their instruction streams in order and Tile semaphores
    only increase, so a wait for (sem, value) when an earlier instruction on
    the same engine already waited for (sem, value' >= value) is a no-op.
    Tile's sem assignment emits these redundant waits at PSUM slot-reuse
    boundaries, where they overflow the LDWEIGHTS format's 1-wait budget.
    """
    skip = ("InstISA", "InstCustomDveAnt", "InstEventSemaphore")
    for b in nc.m.functions[0].blocks:
        seen = {}
        for ins in b.instructions:
            si = ins.sync_info
            if si is None or type(ins).__name__ in skip:
                continue
            w = si.on_wait
            if not w:
                continue
            eng = str(ins.engine).split(".")[-1]
            em = seen.setdefault(eng, {})
            kept = []
            for x in w:
                monotone = x.ant_name is not None and x.ant_name.startswith(
                    ("PE_", "DVE_", "Activation_", "Pool_", "SP_",
                     "DMAHW", "DMASW"))
                if (x.wait_mode != "sem-ge-imm" or x.wait_value is None
                        or x.wait_reg is not None or not monotone):
                    kept.append(x)
                    continue
                # ACT's same-engine self-waits are redundant (in-order
                # engine, writes cannot overtake) — but only drop them when
                # the instruction carries other waits too, to stay within
                # the 1-wait format budget without tripping CoreSim's
                # strict same-engine RAW detector on solo self-waits.
                if (eng == "Activation" and len(w) > 1
                        and x.ant_name.startswith(eng + "_")):
                    continue
                if em.get(x.ant_name, -1) >= x.wait_value:
                    continue
                kept.append(x)
                em[x.ant_name] = x.wait_value
            if len(kept) != len(w):
                si.on_wait = kept


def _split_excess_waits(nc):
    """Move all-but-one sync waits onto injected same-engine NoOps.

    TPB compute instruction formats encode a single embedded wait command
    (walrus rejects more with "Too many sync wait commands"). A NoOp on the
    same in-order engine carrying the extra waits immediately before the
    instruction is semantically identical: the engine stalls at the nop
    until the semaphores reach their thresholds, then proceeds.
    """
    n = 0
    skip = ("InstISA", "InstCustomDveAnt", "InstEventSemaphore")
    for b in nc.m.functions[0].blocks:
        out = []
        changed = False
        for ins in b.instructions:
            si = ins.sync_info
            w = list(si.on_wait) if si is not None and si.on_wait else []
            if len(w) > 1 and type(ins).__name__ not in skip:
                for x in w[:-1]:
                    n += 1
                    nop = mybir.InstNoOp(name=f"I-waitnop-{n}",
                                         engine=ins.engine)
                    nop.sync_info = mybir.SyncInfo(on_wait=[x], on_update=[])
                    try:
                        nop.debug = ins.debug
                    except Exception:
                        pass
                    out.append(nop)
                si.on_wait = [w[-1]]
                changed = True
            out.append(ins)
        if changed:
            b.instructions = out
    return n


def build_program(beta: float, split_waits: bool = True):
    nc = bass.Bass()
    xs = nc.declare_dram_parameter("xs", [SL * SLICE_E + 8], BF16,
                                   isOutput=False)
    lam = nc.declare_dram_parameter("lam", [NT, 128, 1], F32, isOutput=False)
    ident = nc.declare_dram_parameter("ident", [128, 128], BF16,
                                      isOutput=False)
    y = nc.declare_dram_parameter("y", [SL, H, W], F32, isOutput=True)

    taps = [(dy, dx) for dy in range(5) for dx in range(5)]

    with TileContext(nc) as tc:
        with (
            tc.tile_pool(name="consts", bufs=1) as cpool,
            tc.tile_pool(name="main", bufs=2) as pool,
            tc.psum_pool(name="ps", bufs=1) as pspool,
        ):
            itile = cpool.tile([128, 128], BF16)
            nc.sync.dma_start(out=itile[:], in_=ident[:])
            cbias = cpool.tile([128, 1], F32)
            nc.vector.memset(cbias[:], C_BIAS)

            for t in range(NT):
                # ---- load big tile t: even and odd column-parity copies ----
                base = t * SPT * SLICE_E
                dims = [[SLICE_E, SPT], [STRIP_E, NSTRIP], [1, INNER_E]]
                Xe = pool.tile([128, RP, WP], BF16, tag="Xe")
                nc.sync.dma_start(out=Xe[:], in_=_dram_ap(xs, base, dims))
                Xo = pool.tile([128, RP, WP], BF16, tag="Xo")
                nc.sync.dma_start(out=Xo[:], in_=_dram_ap(xs, base + 1, dims))
                lamt = pool.tile([128, 1], F32, tag="lam")
                nc.sync.dma_start(out=lamt[:], in_=lam[t])

                def tap_view(src_e, src_o, dy, dx, r0=0, rows=R):
                    if dx % 2 == 0:
                        return src_e[:, dy + r0:dy + r0 + rows, dx:dx + W]
                    return src_o[:, dy + r0:dy + r0 + rows,
                                 dx - 1:dx - 1 + W]

                # ---- 25-sum (for the mean) on the TensorEngine ----
                # Identity-matmul accumulation into PSUM; borrows the "S"
                # PSUM slot (released before the tap loop needs it).
                Mps = pspool.tile([128, R, W], F32, tag="S")
                for k, (dy, dx) in enumerate(taps):
                    for s in range(4):
                        sel = (slice(None), slice(2 * s, 2 * s + 2),
                               slice(None))
                        vseg = tap_view(Xe, Xo, dy, dx, r0=2 * s, rows=2)
                        nc.tensor.matmul(Mps[sel], itile[:], vseg,
                                         start=(k == 0), stop=(k == 24))
                # nmu = -mean = Mps * (-1/25), evacuated via ACT's free affine
                nmu = pool.tile([128, R, W], BF16, tag="nmu")
                nc.scalar.activation(nmu[:], Mps[:], AF.Copy,
                                     scale=-1.0 / 25.0)

                # ---- separable 5x5 max on DVE (bf16, 2x mode) ----
                MVe = pool.tile([128, R, WP], BF16, tag="MVe")
                nc.vector.tensor_tensor(MVe[:], Xe[:, 0:R, :],
                                        Xe[:, 1:R + 1, :], ALU.max)
                for dy in (2, 3, 4):
                    nc.vector.tensor_tensor(MVe[:], MVe[:],
                                            Xe[:, dy:dy + R, :], ALU.max)
                MVo = pool.tile([128, R, WP], BF16, tag="MVo")
                nc.vector.tensor_tensor(MVo[:], Xo[:, 0:R, :],
                                        Xo[:, 1:R + 1, :], ALU.max)
                for dy in (2, 3, 4):
                    nc.vector.tensor_tensor(MVo[:], MVo[:],
                                            Xo[:, dy:dy + R, :], ALU.max)
                MX = pool.tile([128, R, W], BF16, tag="MX")
                nc.vector.tensor_tensor(MX[:], MVe[:, :, 0:W],
                                        MVe[:, :, 2:W + 2], ALU.max)
                nc.vector.tensor_tensor(MX[:], MX[:], MVe[:, :, 4:W + 4],
                                        ALU.max)
                nc.vector.tensor_tensor(MX[:], MX[:], MVo[:, :, 0:W],
                                        ALU.max)
                nc.vector.tensor_tensor(MX[:], MX[:], MVo[:, :, 2:W + 2],
                                        ALU.max)

                # ---- 25 taps: softmax-weighted sums ----
                S_ps = pspool.tile([128, R, W], F32, tag="S")
                T_ps = pspool.tile([128, R, W], F32, tag="T")
                for k, (dy, dx) in enumerate(taps):
                    v = tap_view(Xe, Xo, dy, dx)
                    d = pool.tile([128, R, W], BF16, tag="d")
                    nc.vector.tensor_tensor(d[:], v, nmu[:], ALU.add)
                    # |d| in place: clear the bf16 sign bit via an int16
                    # bitcast (valid TS op, 4x mode)
                    di = d[:].bitcast(mybir.dt.int16)
                    nc.vector.tensor_scalar(di, di, 0x7FFF, None,
                                            ALU.bitwise_and)
                    st = pool.tile([128, R, W], BF16, tag="st")
                    nc.scalar.activation(st[:], d[:], AF.Exp,
                                         bias=cbias[:], scale=-beta)
                    pt = pool.tile([128, R, W], BF16, tag="pt")
                    nc.vector.tensor_tensor(pt[:], st[:], v, ALU.mult)
                    last = k == 24
                    for s in range(4):
                        sel = (slice(None), slice(2 * s, 2 * s + 2),
                               slice(None))
                        nc.tensor.matmul(S_ps[sel], itile[:], st[sel],
                                         start=(k == 0), stop=last)
                        nc.tensor.matmul(T_ps[sel], itile[:], pt[sel],
                                         start=(k == 0), stop=last)

                # ---- combine (f32): out = lam*(T/S) + (1-lam)*max ----
                lnS = pool.tile([128, R, W], F32, tag="lnS")
                nc.scalar.activation(lnS[:], S_ps[:], AF.Ln)
                rS = pool.tile([128, R, W], F32, tag="rS")
                nc.scalar.activation(rS[:], lnS[:], AF.Exp, scale=-1.0)
                med = pool.tile([128, R, W], F32, tag="med")
                nc.vector.tensor_tensor(med[:], rS[:], T_ps[:], ALU.mult)
                dif = pool.tile([128, R, W], F32, tag="dif")
                nc.vector.tensor_tensor(dif[:], med[:], MX[:], ALU.subtract)
                q = pool.tile([128, R, W], F32, tag="q")
                nc.vector.tensor_scalar(q[:], dif[:], lamt[:], None, ALU.mult)
                out_t = pool.tile([128, R, W], F32, tag="out")
                nc.vector.tensor_tensor(out_t[:], q[:], MX[:], ALU.add)

                nc.sync.dma_start(
                    out=_dram_ap(y, t * SPT * HW,
                                 [[HW, SPT], [R * W, NSTRIP], [1, R * W]]),
                    in_=out_t[:],
                )
    _elide_covered_waits(nc)
    if split_waits:
        # Mechanical transform for walrus's 1-wait instruction formats;
        # skip under CoreSim (its race detector requires sem updates on
        # every instruction, which the injected bare NoOps lack).
        _split_excess_waits(nc)
    return nc


def _make_inputs(x, mix, beta_raw):
    """Host-side sharding. Returns (beta, in_maps)."""
    x = np.ascontiguousarray(x, dtype=np.float32)
    mix = np.asarray(mix, dtype=np.float32).reshape(C)
    beta_raw = float(np.asarray(beta_raw, dtype=np.float32))
    beta = float(5.0 + 45.0 / (1.0 + np.exp(-beta_raw)))
    lam_c = (1.0 / (1.0 + np.exp(-mix.astype(np.float64)))).astype(np.float32)

    xs_all = np.pad(x.reshape(B * C, H, W), ((0, 0), (2, 2), (2, 4)),
                    mode="reflect").astype(ml_dtypes.bfloat16)
    ident = np.eye(128, dtype=ml_dtypes.bfloat16)
    in_maps = []
    for core in range(NCORES):
        sl0 = core * SL
        shard = np.zeros(SL * SLICE_E + 8, dtype=ml_dtypes.bfloat16)
        shard[:SL * SLICE_E] = xs_all[sl0:sl0 + SL].reshape(-1)
        lam_t = np.empty((NT, 128, 1), dtype=np.float32)
        for t in range(NT):
            for p in range(128):
                g_slice = sl0 + t * SPT + p // NSTRIP
                lam_t[t, p, 0] = lam_c[g_slice % C]
        in_maps.append({"xs": shard, "lam": lam_t, "ident": ident})
    return beta, in_maps


def kernel(x, mix, beta_raw):
    beta, in_maps = _make_inputs(x, mix, beta_raw)
    nc = build_program(beta)
    res = run_bass_kernel_spmd(nc, in_maps, list(range(NCORES))).results
    out = np.concatenate([res[i]["y"].reshape(SL, H, W)
                          for i in range(NCORES)], axis=0)
    return np.ascontiguousarray(out.reshape(B, C, H, W))


# revision 10
# speedup vs baseline: 1.7400x; 1.0140x over previous
"""Trainium2 Bass kernel for DifferentiableMaxMedian (5x5 reflect-padded
max filter + soft-median filter, per-channel mix).

Contract: kernel(**inputs) takes FULL numpy inputs
  x: (4,32,256,256) f32, mix: (1,32,1,1) f32, beta_raw: () f32
and returns the full (4,32,256,256) f32 output.

Sharding: pure data parallel over B*C = 128 (b,c) slices -> 16 slices/core
across 8 cores.

Per-core layout: each slice is reflect-padded host-side to 260x262 in BF16
and cut into 32 row-strips of R=8 output rows. 4 slices x 32 strips = 128
SBUF partitions per big tile (4 big tiles/core). A partition's free dim
holds its strip's 12 rows (8 + 4 halo) x 262 padded cols, contiguous in
DRAM, so each big tile loads with ONE DMA per column-parity copy and every
5x5 window tap is a pure free-dim AP offset. Two copies (even / odd column
base) keep every 16-bit DVE operand 4-byte aligned, which is what unlocks
the 2x_1P DVE mode for the hot per-tap tensor_tensor ops.

Math per tap k: s_k = exp(C - beta*|v_k - mu|); the softmax ratio is
invariant to C. The 25-tap window mean 25*mu is accumulated on the
TensorEngine as identity-matmul PSUM accumulations and evacuated through
the Scalar engine with a free -1/25 affine (nmu = -mu). S = sum_k s_k and
T = sum_k s_k*v_k accumulate on the TensorEngine in BF16 (PSUM keeps f32).
The combine runs in f32: out = lam*(T/S) + (1-lam)*max5x5, with
1/S = exp(-ln S) on ACT.
"""

import numpy as np
import ml_dtypes

from concourse import bass
import concourse.mybir as mybir
import concourse.tile_sem_assignment as _tsa
from concourse.ap import AP
from concourse.bass_utils import run_bass_kernel_spmd
from concourse.tile import TileContext
from concourse.mybir import AluOpType as ALU
from concourse.mybir import ActivationFunctionType as AF

# All our DMAs issue from the sync engine's single HW-DGE ring (FIFO
# completion), so one bookkeeping sem lane is sound — and it keeps
# per-instruction wait counts under the per-engine ISA limits.
_tsa.NUM_HWDGE_SEMS = 1

F32 = mybir.dt.float32
BF16 = mybir.dt.bfloat16

B, C, H, W = 4, 32, 256, 256
NCORES = 8
SL = (B * C) // NCORES     # 16 slices per core
R = 8                      # output rows per strip
RP = R + 4                 # rows incl halo
HP = H + 4                 # padded rows per slice
WP = W + 6                 # padded width (2 left, 4 right: odd-shift slack)
NSTRIP = H // R            # 32 strips per slice
SPT = 128 // NSTRIP        # 4 slices per big tile
NT = SL // SPT             # 4 big tiles per core
# exp stability bias (cancels in the softmax ratio). Keeps S = sum_k
# exp(C - beta*|d_k|) below the ACT-Ln valid range (< 2^64) while making
# S-underflow impossible for any window realizable from N(0,1) data.
C_BIAS = 40.0

SLICE_E = HP * WP          # elements per padded slice
STRIP_E = R * WP           # strip stride in elements
INNER_E = RP * WP          # elements per partition load
HW = H * W


def _dram_ap(t, offset, dims):
    return AP(tensor=t[:].tensor, offset=offset, ap=dims)


def _elide_covered_waits(nc):
    """Drop sem waits already covered by an earlier wait on the same engine.

    Engines execute their instruction streams in order and Tile semaphores
    only increase, so a wait for (sem, value) when an earlier instruction on
    the same engine already waited for (sem, value' >= value) is a no-op.
    Tile's sem assignment emits these redundant waits at PSUM slot-reuse
    boundaries, where they overflow the LDWEIGHTS format's 1-wait budget.
    """
    skip = ("InstISA", "InstCustomDveAnt", "InstEventSemaphore")
    for b in nc.m.functions[0].blocks:
        seen = {}
        for ins in b.instructions:
            si = ins.sync_info
            if si is None or type(ins).__name__ in skip:
                continue
            w = si.on_wait
            if not w:
                continue
            eng = str(ins.engine).split(".")[-1]
            em = seen.setdefault(eng, {})
            kept = []
            for x in w:
                monotone = x.ant_name is not None and x.ant_name.startswith(
                    ("PE_", "DVE_", "Activation_", "Pool_", "SP_",
                     "DMAHW", "DMASW"))
                if (x.wait_mode != "sem-ge-imm" or x.wait_value is None
                        or x.wait_reg is not None or not monotone):
                    kept.append(x)
                    continue
                # ACT's same-engine self-waits are redundant (in-order
                # engine, writes cannot overtake) — but only drop them when
                # the instruction carries other waits too, to stay within
                # the 1-wait format budget without tripping CoreSim's
                # strict same-engine RAW detector on solo self-waits.
                if (eng == "Activation" and len(w) > 1
                        and x.ant_name.startswith(eng + "_")):
                    continue
                if em.get(x.ant_name, -1) >= x.wait_value:
                    continue
                kept.append(x)
                em[x.ant_name] = x.wait_value
            if len(kept) != len(w):
                si.on_wait = kept


def _split_excess_waits(nc):
    """Move all-but-one sync waits onto injected same-engine NoOps.

    TPB compute instruction formats encode a single embedded wait command
    (walrus rejects more with "Too many sync wait commands"). A NoOp on the
    same in-order engine carrying the extra waits immediately before the
    instruction is semantically identical: the engine stalls at the nop
    until the semaphores reach their thresholds, then proceeds.
    """
    n = 0
    skip = ("InstISA", "InstCustomDveAnt", "InstEventSemaphore")
    for b in nc.m.functions[0].blocks:
        out = []
        changed = False
        for ins in b.instructions:
            si = ins.sync_info
            w = list(si.on_wait) if si is not None and si.on_wait else []
            if len(w) > 1 and type(ins).__name__ not in skip:
                for x in w[:-1]:
                    n += 1
                    nop = mybir.InstNoOp(name=f"I-waitnop-{n}",
                                         engine=ins.engine)
                    nop.sync_info = mybir.SyncInfo(on_wait=[x], on_update=[])
                    try:
                        nop.debug = ins.debug
                    except Exception:
                        pass
                    out.append(nop)
                si.on_wait = [w[-1]]
                changed = True
            out.append(ins)
        if changed:
            b.instructions = out
    return n


ABS_ON_GPSIMD = False  # walrus: TensorScalar is not a valid Pool-engine op


def build_program(beta: float, split_waits: bool = True):
    nc = bass.Bass()
    xs = nc.declare_dram_parameter("xs", [SL * SLICE_E + 8], BF16,
                                   isOutput=False)
    lam = nc.declare_dram_parameter("lam", [NT, 128, 1], F32, isOutput=False)
    ident = nc.declare_dram_parameter("ident", [128, 128], BF16,
                                      isOutput=False)
    y = nc.declare_dram_parameter("y", [SL, H, W], F32, isOutput=True)

    taps = [(dy, dx) for dy in range(5) for dx in range(5)]

    with TileContext(nc) as tc:
        with (
            tc.tile_pool(name="consts", bufs=1) as cpool,
            tc.tile_pool(name="main", bufs=2) as pool,
            tc.psum_pool(name="ps", bufs=1) as pspool,
        ):
            itile = cpool.tile([128, 128], BF16)
            nc.sync.dma_start(out=itile[:], in_=ident[:])
            cbias = cpool.tile([128, 1], F32)
            nc.vector.memset(cbias[:], C_BIAS)

            X = {}     # t -> stacked [128, 2, RP, WP] even/odd tile
            LAM = {}   # t -> [128, 1] per-partition lambda
            MV = {}    # t -> vertical-max partial [128, 2, R, WP]
            MXs = {}   # t -> 5x5 max [128, R, W]
            NMU = {}   # t -> -mean [128, R, W]

            def load_tile(t):
                base = t * SPT * SLICE_E
                dims = [[SLICE_E, SPT], [STRIP_E, NSTRIP], [1, INNER_E]]
                Xt = pool.tile([128, 2, RP, WP], BF16, tag="X", bufs=3)
                nc.sync.dma_start(out=Xt[:, 0], in_=_dram_ap(xs, base, dims))
                nc.sync.dma_start(out=Xt[:, 1],
                                  in_=_dram_ap(xs, base + 1, dims))
                lamt = pool.tile([128, 1], F32, tag="lam", bufs=3)
                nc.sync.dma_start(out=lamt[:], in_=lam[t])
                X[t], LAM[t] = Xt, lamt

            def tap_view(t, dy, dx, r0=0, rows=R):
                par = dx % 2
                c0 = dx - par   # column base within that parity plane
                return X[t][:, par, dy + r0:dy + r0 + rows, c0:c0 + W]

            def vmax_tree(t):
                # vertical 5-max over both parity planes in one stream
                Xt = X[t]
                MVt = pool.tile([128, 2, R, WP], BF16, tag="MV", bufs=2)
                nc.vector.tensor_tensor(MVt[:], Xt[:, :, 0:R, :],
                                        Xt[:, :, 1:R + 1, :], ALU.max)
                for dy in (2, 3, 4):
                    nc.vector.tensor_tensor(MVt[:], MVt[:],
                                            Xt[:, :, dy:dy + R, :], ALU.max)
                MV[t] = MVt

            def hmax(t):
                MVt = MV[t]
                MXt = pool.tile([128, R, W], BF16, tag="MX", bufs=2)
                nc.vector.tensor_tensor(MXt[:], MVt[:, 0, :, 0:W],
                                        MVt[:, 0, :, 2:W + 2], ALU.max)
                nc.vector.tensor_tensor(MXt[:], MXt[:],
                                        MVt[:, 0, :, 4:W + 4], ALU.max)
                nc.vector.tensor_tensor(MXt[:], MXt[:],
                                        MVt[:, 1, :, 0:W], ALU.max)
                nc.vector.tensor_tensor(MXt[:], MXt[:],
                                        MVt[:, 1, :, 2:W + 2], ALU.max)
                MXs[t] = MXt

            def mean_dve(t):
                # tile-0 only: 25-sum on DVE while the PE is still cold
                Xt = X[t]
                Veo = pool.tile([128, 2, R, WP], BF16, tag="Veo", bufs=1)
                nc.vector.tensor_tensor(Veo[:], Xt[:, :, 0:R, :],
                                        Xt[:, :, 1:R + 1, :], ALU.add)
                for dy in (2, 3, 4):
                    nc.vector.tensor_tensor(Veo[:], Veo[:],
                                            Xt[:, :, dy:dy + R, :], ALU.add)
                Ms = pool.tile([128, R, W], BF16, tag="Ms", bufs=1)
                nc.vector.tensor_tensor(Ms[:], Veo[:, 0, :, 0:W],
                                        Veo[:, 0, :, 2:W + 2], ALU.add)
                nc.vector.tensor_tensor(Ms[:], Ms[:], Veo[:, 0, :, 4:W + 4],
                                        ALU.add)
                nc.vector.tensor_tensor(Ms[:], Ms[:], Veo[:, 1, :, 0:W],
                                        ALU.add)
                nc.vector.tensor_tensor(Ms[:], Ms[:], Veo[:, 1, :, 2:W + 2],
                                        ALU.add)
                nmu = pool.tile([128, R, W], BF16, tag="nmu", bufs=2)
                nc.vector.tensor_scalar(nmu[:], Ms[:], -1.0 / 25.0, None,
                                        ALU.mult)
                NMU[t] = nmu

            def mean_pe(t):
                # 25-sum as identity-matmul PSUM accumulation; borrows the
                # "S" slot (free between ln(S_{t-1}) and the tap loop of t).
                Mps = pspool.tile([128, R, W], F32, tag="S")
                for k, (dy, dx) in enumerate(taps):
                    for s in range(4):
                        sel = (slice(None), slice(2 * s, 2 * s + 2),
                               slice(None))
                        vseg = tap_view(t, dy, dx, r0=2 * s, rows=2)
                        nc.tensor.matmul(Mps[sel], itile[:], vseg,
                                         start=(k == 0), stop=(k == 24))
                # nmu = -mean: evacuate with ACT's free -1/25 affine
                nmu = pool.tile([128, R, W], BF16, tag="nmu", bufs=2)
                nc.scalar.activation(nmu[:], Mps[:], AF.Copy,
                                     scale=-1.0 / 25.0)
                NMU[t] = nmu

            # ---- prologue ----
            load_tile(0)
            load_tile(1)
            mean_dve(0)
            vmax_tree(0)
            hmax(0)
            vmax_tree(1)

            for t in range(NT):
                # ---- 25 taps: softmax-weighted sums ----
                nmu = NMU[t]
                S_ps = pspool.tile([128, R, W], F32, tag="S")
                T_ps = pspool.tile([128, R, W], F32, tag="T")
                for k, (dy, dx) in enumerate(taps):
                    v = tap_view(t, dy, dx)
                    d = pool.tile([128, R, W], BF16, tag="d", bufs=3)
                    nc.vector.tensor_tensor(d[:], v, nmu[:], ALU.add)
                    # |d| in place: clear the bf16 sign bit via an int16
                    # bitcast (single-src TS, 4x mode on DVE)
                    di = d[:].bitcast(mybir.dt.int16)
                    eng = nc.gpsimd if ABS_ON_GPSIMD else nc.vector
                    eng.tensor_scalar(di, di, 0x7FFF, None, ALU.bitwise_and)
                    st = pool.tile([128, R, W], BF16, tag="st", bufs=3)
                    nc.scalar.activation(st[:], d[:], AF.Exp,
                                         bias=cbias[:], scale=-beta)
                    pt = pool.tile([128, R, W], BF16, tag="pt", bufs=3)
                    nc.vector.tensor_tensor(pt[:], st[:], v, ALU.mult)
                    last = k == 24
                    for s in range(4):
                        sel = (slice(None), slice(2 * s, 2 * s + 2),
                               slice(None))
                        nc.tensor.matmul(S_ps[sel], itile[:], st[sel],
                                         start=(k == 0), stop=last)
                        nc.tensor.matmul(T_ps[sel], itile[:], pt[sel],
                                         start=(k == 0), stop=last)

                # prefetch t+2 and give the DVE deferred filler work that
                # overlaps the PE mean phase of tile t+1 below
                if t + 2 < NT:
                    load_tile(t + 2)
                    vmax_tree(t + 2)

                # ---- combine (f32): out = lam*(T/S) + (1-lam)*max ----
                lnS = pool.tile([128, R, W], F32, tag="lnS")
                nc.scalar.activation(lnS[:], S_ps[:], AF.Ln)
                rS = pool.tile([128, R, W], F32, tag="rS")
                nc.scalar.activation(rS[:], lnS[:], AF.Exp, scale=-1.0)
                MXt = MXs.pop(t)
                med = pool.tile([128, R, W], F32, tag="med")
                nc.vector.tensor_tensor(med[:], rS[:], T_ps[:], ALU.mult)
                nc.vector.tensor_tensor(med[:], med[:], MXt[:], ALU.subtract)
                nc.vector.tensor_scalar(med[:], med[:], LAM[t][:], None,
                                        ALU.mult)
                out_t = pool.tile([128, R, W], F32, tag="out")
                nc.vector.tensor_tensor(out_t[:], med[:], MXt[:], ALU.add)
                nc.sync.dma_start(
                    out=_dram_ap(y, t * SPT * HW,
                                 [[HW, SPT], [R * W, NSTRIP], [1, R * W]]),
                    in_=out_t[:],
                )
                X.pop(t)

                # ---- mean + horizontal max for tile t+1 (fills the gap
                # between ln(S_t) and the next tap loop) ----
                if t + 1 < NT:
                    hmax(t + 1)
                    mean_pe(t + 1)
    _elide_covered_waits(nc)
    if split_waits:
        # Mechanical transform for walrus's 1-wait instruction formats;
        # skip under CoreSim (its race detector requires sem updates on
        # every instruction, which the injected bare NoOps lack).
        _split_excess_waits(nc)
    return nc


def _make_inputs(x, mix, beta_raw):
    """Host-side sharding. Returns (beta, in_maps)."""
    x = np.ascontiguousarray(x, dtype=np.float32)
    mix = np.asarray(mix, dtype=np.float32).reshape(C)
    beta_raw = float(np.asarray(beta_raw, dtype=np.float32))
    beta = float(5.0 + 45.0 / (1.0 + np.exp(-beta_raw)))
    lam_c = (1.0 / (1.0 + np.exp(-mix.astype(np.float64)))).astype(np.float32)

    xs_all = np.pad(x.reshape(B * C, H, W), ((0, 0), (2, 2), (2, 4)),
                    mode="reflect").astype(ml_dtypes.bfloat16)
    ident = np.eye(128, dtype=ml_dtypes.bfloat16)
    in_maps = []
    for core in range(NCORES):
        sl0 = core * SL
        shard = np.zeros(SL * SLICE_E + 8, dtype=ml_dtypes.bfloat16)
        shard[:SL * SLICE_E] = xs_all[sl0:sl0 + SL].reshape(-1)
        lam_t = np.empty((NT, 128, 1), dtype=np.float32)
        for t in range(NT):
            for p in range(128):
                g_slice = sl0 + t * SPT + p // NSTRIP
                lam_t[t, p, 0] = lam_c[g_slice % C]
        in_maps.append({"xs": shard, "lam": lam_t, "ident": ident})
    return beta, in_maps


def kernel(x, mix, beta_raw):
    beta, in_maps = _make_inputs(x, mix, beta_raw)
    nc = build_program(beta)
    res = run_bass_kernel_spmd(nc, in_maps, list(range(NCORES))).results
    out = np.concatenate([res[i]["y"].reshape(SL, H, W)
                          for i in range(NCORES)], axis=0)
    return np.ascontiguousarray(out.reshape(B, C, H, W))


# revision 21
# speedup vs baseline: 1.8820x; 1.0816x over previous
"""Trainium2 Bass kernel for DifferentiableMaxMedian (5x5 reflect-padded
max filter + soft-median filter, per-channel mix).

Contract: kernel(**inputs) takes FULL numpy inputs
  x: (4,32,256,256) f32, mix: (1,32,1,1) f32, beta_raw: () f32
and returns the full (4,32,256,256) f32 output.

Sharding: pure data parallel over B*C = 128 (b,c) slices -> 16 slices/core
across 8 cores.

Per-core layout: each slice is reflect-padded host-side to 260x262 in BF16
and cut into 32 row-strips of R=8 output rows. 4 slices x 32 strips = 128
SBUF partitions per big tile (4 big tiles/core). A partition's free dim
holds its strip's 12 rows (8 + 4 halo) x 262 padded cols, contiguous in
DRAM, so each big tile loads with ONE DMA per column-parity copy and every
5x5 window tap is a pure free-dim AP offset. Two copies (even / odd column
base) keep every 16-bit DVE operand 4-byte aligned, which is what unlocks
the 2x_1P DVE mode for the hot per-tap tensor_tensor ops.

Math per tap k: s_k = exp(C - beta*|v_k - mu|); the softmax ratio is
invariant to C. The 25-tap window mean 25*mu is accumulated on the
TensorEngine as identity-matmul PSUM accumulations and evacuated through
the Scalar engine with a free -1/25 affine (nmu = -mu). S = sum_k s_k and
T = sum_k s_k*v_k accumulate on the TensorEngine in BF16 (PSUM keeps f32).
The combine runs in f32: out = lam*(T/S) + (1-lam)*max5x5, with
1/S = exp(-ln S) on ACT.
"""

import numpy as np
import ml_dtypes

from concourse import bass
import concourse.mybir as mybir
import concourse.tile_sem_assignment as _tsa
from concourse.ap import AP
from concourse.bass_utils import run_bass_kernel_spmd
from concourse.tile import TileContext
from concourse.tile_rust import add_dep_helper
from concourse.mybir import AluOpType as ALU
from concourse.mybir import ActivationFunctionType as AF

# All our DMAs issue from the sync engine's single HW-DGE ring (FIFO
# completion), so one bookkeeping sem lane is sound — and it keeps
# per-instruction wait counts under the per-engine ISA limits.
_tsa.NUM_HWDGE_SEMS = 1

F32 = mybir.dt.float32
BF16 = mybir.dt.bfloat16

B, C, H, W = 4, 32, 256, 256
NCORES = 8
SL = (B * C) // NCORES     # 16 slices per core
R = 8                      # output rows per strip
RP = R + 4                 # rows incl halo
HP = H + 4                 # padded rows per slice
WP = W + 6                 # padded width (2 left, 4 right: odd-shift slack)
NSTRIP = H // R            # 32 strips per slice
SPT = 128 // NSTRIP        # 4 slices per big tile
NT = SL // SPT             # 4 big tiles per core
# exp stability bias (cancels in the softmax ratio). Keeps S = sum_k
# exp(C - beta*|d_k|) below the ACT-Ln valid range (< 2^64) while making
# S-underflow impossible for any window realizable from N(0,1) data.
C_BIAS = 40.0

SLICE_E = HP * WP          # elements per padded slice
STRIP_E = R * WP           # strip stride in elements
INNER_E = RP * WP          # elements per partition load
HW = H * W


def _dram_ap(t, offset, dims):
    return AP(tensor=t[:].tensor, offset=offset, ap=dims)


def _elide_covered_waits(nc):
    """Drop sem waits already covered by an earlier wait on the same engine.

    Engines execute their instruction streams in order and Tile semaphores
    only increase, so a wait for (sem, value) when an earlier instruction on
    the same engine already waited for (sem, value' >= value) is a no-op.
    Tile's sem assignment emits these redundant waits at PSUM slot-reuse
    boundaries, where they overflow the LDWEIGHTS format's 1-wait budget.
    """
    skip = ("InstISA", "InstCustomDveAnt", "InstEventSemaphore")
    for b in nc.m.functions[0].blocks:
        seen = {}
        for ins in b.instructions:
            si = ins.sync_info
            if si is None or type(ins).__name__ in skip:
                continue
            w = si.on_wait
            if not w:
                continue
            eng = str(ins.engine).split(".")[-1]
            em = seen.setdefault(eng, {})
            kept = []
            for x in w:
                monotone = x.ant_name is not None and x.ant_name.startswith(
                    ("PE_", "DVE_", "Activation_", "Pool_", "SP_",
                     "DMAHW", "DMASW"))
                if (x.wait_mode != "sem-ge-imm" or x.wait_value is None
                        or x.wait_reg is not None or not monotone):
                    kept.append(x)
                    continue
                # ACT's same-engine self-waits are redundant (in-order
                # engine, writes cannot overtake) — but only drop them when
                # the instruction carries other waits too, to stay within
                # the 1-wait format budget without tripping CoreSim's
                # strict same-engine RAW detector on solo self-waits.
                if (eng == "Activation" and len(w) > 1
                        and x.ant_name.startswith(eng + "_")):
                    continue
                if em.get(x.ant_name, -1) >= x.wait_value:
                    continue
                kept.append(x)
                em[x.ant_name] = x.wait_value
            if len(kept) != len(w):
                si.on_wait = kept


def _split_excess_waits(nc):
    """Move all-but-one sync waits onto injected same-engine NoOps.

    TPB compute instruction formats encode a single embedded wait command
    (walrus rejects more with "Too many sync wait commands"). A NoOp on the
    same in-order engine carrying the extra waits immediately before the
    instruction is semantically identical: the engine stalls at the nop
    until the semaphores reach their thresholds, then proceeds.
    """
    n = 0
    skip = ("InstISA", "InstCustomDveAnt", "InstEventSemaphore")
    for b in nc.m.functions[0].blocks:
        out = []
        changed = False
        for ins in b.instructions:
            si = ins.sync_info
            w = list(si.on_wait) if si is not None and si.on_wait else []
            if len(w) > 1 and type(ins).__name__ not in skip:
                for x in w[:-1]:
                    n += 1
                    nop = mybir.InstNoOp(name=f"I-waitnop-{n}",
                                         engine=ins.engine)
                    nop.sync_info = mybir.SyncInfo(on_wait=[x], on_update=[])
                    try:
                        nop.debug = ins.debug
                    except Exception:
                        pass
                    out.append(nop)
                si.on_wait = [w[-1]]
                changed = True
            out.append(ins)
        if changed:
            b.instructions = out
    return n


ABS_ON_GPSIMD = False  # walrus: TensorScalar is not a valid Pool-engine op


def build_program(beta: float, split_waits: bool = True):
    nc = bass.Bass()
    xs = nc.declare_dram_parameter("xs", [SL * SLICE_E + 8], BF16,
                                   isOutput=False)
    lam = nc.declare_dram_parameter("lam", [NT, 128, 1], F32, isOutput=False)
    ident = nc.declare_dram_parameter("ident", [128, 128], BF16,
                                      isOutput=False)
    y = nc.declare_dram_parameter("y", [SL, H, W], F32, isOutput=True)

    taps = [(dy, dx) for dy in range(5) for dx in range(5)]

    with TileContext(nc) as tc:
        with (
            tc.tile_pool(name="consts", bufs=1) as cpool,
            tc.tile_pool(name="main", bufs=2) as pool,
            tc.psum_pool(name="ps", bufs=1) as pspool,
        ):
            itile = cpool.tile([128, 128], BF16)
            cbias = cpool.tile([128, 1], F32)
            nc.vector.memset(cbias[:], C_BIAS)

            X = {}     # t -> stacked [128, 2, RP, WP] even/odd tile
            LAM = {}   # t -> [128, 1] per-partition lambda
            MV = {}    # t -> vertical-max partial [128, 2, R, WP]
            MXs = {}   # t -> 5x5 max [128, R, W]
            NMU = {}   # t -> -mean [128, R, W]

            def load_tile(t):
                base = t * SPT * SLICE_E
                dims = [[SLICE_E, SPT], [STRIP_E, NSTRIP], [1, INNER_E]]
                Xt = pool.tile([128, 2, RP, WP], BF16, tag="X", bufs=3)
                nc.sync.dma_start(out=Xt[:, 0], in_=_dram_ap(xs, base, dims))
                nc.sync.dma_start(out=Xt[:, 1],
                                  in_=_dram_ap(xs, base + 1, dims))
                lamt = pool.tile([128, 1], F32, tag="lam", bufs=3)
                nc.sync.dma_start(out=lamt[:], in_=lam[t])
                X[t], LAM[t] = Xt, lamt

            def tap_view(t, dy, dx, r0=0, rows=R):
                par = dx % 2
                c0 = dx - par   # column base within that parity plane
                return X[t][:, par, dy + r0:dy + r0 + rows, c0:c0 + W]

            def vmax_tree(t):
                # vertical 5-max over both parity planes in one stream
                Xt = X[t]
                MVt = pool.tile([128, 2, R, WP], BF16, tag="MV", bufs=2)
                first = nc.vector.tensor_tensor(MVt[:], Xt[:, :, 0:R, :],
                                                Xt[:, :, 1:R + 1, :], ALU.max)
                for dy in (2, 3, 4):
                    nc.vector.tensor_tensor(MVt[:], MVt[:],
                                            Xt[:, :, dy:dy + R, :], ALU.max)
                MV[t] = MVt
                return first

            def hmax(t):
                MVt = MV[t]
                MXt = pool.tile([128, R, W], BF16, tag="MX", bufs=2)
                first = nc.vector.tensor_tensor(MXt[:], MVt[:, 0, :, 0:W],
                                                MVt[:, 0, :, 2:W + 2],
                                                ALU.max)
                nc.vector.tensor_tensor(MXt[:], MXt[:],
                                        MVt[:, 0, :, 4:W + 4], ALU.max)
                nc.vector.tensor_tensor(MXt[:], MXt[:],
                                        MVt[:, 1, :, 0:W], ALU.max)
                nc.vector.tensor_tensor(MXt[:], MXt[:],
                                        MVt[:, 1, :, 2:W + 2], ALU.max)
                MXs[t] = MXt
                return first

            def mean_dve(t):
                # tile-0 only: 25-sum on DVE while the PE is still cold
                Xt = X[t]
                Veo = pool.tile([128, 2, R, WP], BF16, tag="Veo", bufs=1)
                nc.vector.tensor_tensor(Veo[:], Xt[:, :, 0:R, :],
                                        Xt[:, :, 1:R + 1, :], ALU.add)
                for dy in (2, 3, 4):
                    nc.vector.tensor_tensor(Veo[:], Veo[:],
                                            Xt[:, :, dy:dy + R, :], ALU.add)
                Ms = pool.tile([128, R, W], BF16, tag="Ms", bufs=1)
                nc.vector.tensor_tensor(Ms[:], Veo[:, 0, :, 0:W],
                                        Veo[:, 0, :, 2:W + 2], ALU.add)
                nc.vector.tensor_tensor(Ms[:], Ms[:], Veo[:, 0, :, 4:W + 4],
                                        ALU.add)
                nc.vector.tensor_tensor(Ms[:], Ms[:], Veo[:, 1, :, 0:W],
                                        ALU.add)
                nc.vector.tensor_tensor(Ms[:], Ms[:], Veo[:, 1, :, 2:W + 2],
                                        ALU.add)
                nmu = pool.tile([128, R, W], BF16, tag="nmu", bufs=2)
                nc.vector.tensor_scalar(nmu[:], Ms[:], -1.0 / 25.0, None,
                                        ALU.mult)
                NMU[t] = nmu

            def mean_pe(t):
                # 25-sum as identity-matmul PSUM accumulation; borrows the
                # "S" slot (free between ln(S_{t-1}) and the tap loop of t).
                # Segment-major order + per-segment nmu evacuation lets the
                # next tap loop's (segmented) first d-add start after one
                # segment (~7us) instead of after the whole phase (~23us).
                Mps = pspool.tile([128, R, W], F32, tag="S")
                nmu = pool.tile([128, R, W], BF16, tag="nmu", bufs=2)
                first_mm = None
                for s in range(4):
                    sel = (slice(None), slice(2 * s, 2 * s + 2),
                           slice(None))
                    for k, (dy, dx) in enumerate(taps):
                        vseg = tap_view(t, dy, dx, r0=2 * s, rows=2)
                        mm = nc.tensor.matmul(Mps[sel], itile[:], vseg,
                                              start=(k == 0), stop=(k == 24))
                        if first_mm is None:
                            first_mm = mm
                    nc.scalar.activation(nmu[sel], Mps[sel], AF.Copy,
                                         scale=-1.0 / 25.0)
                NMU[t] = nmu
                return first_mm

            # ---- prologue (tile-0 data first; ident is not needed until
            # the first mean matmuls ~40us in) ----
            load_tile(0)
            load_tile(1)
            nc.sync.dma_start(out=itile[:], in_=ident[:])
            mean_dve(0)
            vmax_tree(0)
            hmax(0)
            vmax_tree(1)

            for t in range(NT):
                # ---- 25 taps: softmax-weighted sums ----
                nmu = NMU[t]
                S_ps = pspool.tile([128, R, W], F32, tag="S")
                T_ps = pspool.tile([128, R, W], F32, tag="T")
                last_smm = None
                for k, (dy, dx) in enumerate(taps):
                    v = tap_view(t, dy, dx)
                    d = pool.tile([128, R, W], BF16, tag="d", bufs=3)
                    if k == 0 and t > 0:
                        # segmented: each piece only needs its nmu segment,
                        # which the segment-major mean phase delivers early
                        for s in range(4):
                            sel = (slice(None), slice(2 * s, 2 * s + 2),
                                   slice(None))
                            nc.vector.tensor_tensor(
                                d[sel], tap_view(t, dy, dx, r0=2 * s, rows=2),
                                nmu[sel], ALU.add)
                    else:
                        nc.vector.tensor_tensor(d[:], v, nmu[:], ALU.add)
                    # |d| in place: clear the bf16 sign bit via an int16
                    # bitcast (single-src TS, 4x mode on DVE)
                    di = d[:].bitcast(mybir.dt.int16)
                    eng = nc.gpsimd if ABS_ON_GPSIMD else nc.vector
                    eng.tensor_scalar(di, di, 0x7FFF, None, ALU.bitwise_and)
                    st = pool.tile([128, R, W], BF16, tag="st", bufs=3)
                    nc.scalar.activation(st[:], d[:], AF.Exp,
                                         bias=cbias[:], scale=-beta)
                    pt = pool.tile([128, R, W], BF16, tag="pt", bufs=3)
                    nc.vector.tensor_tensor(pt[:], st[:], v, ALU.mult)
                    last = k == 24
                    for s in range(4):
                        sel = (slice(None), slice(2 * s, 2 * s + 2),
                               slice(None))
                        smm = nc.tensor.matmul(S_ps[sel], itile[:], st[sel],
                                               start=(k == 0), stop=last)
                        nc.tensor.matmul(T_ps[sel], itile[:], pt[sel],
                                         start=(k == 0), stop=last)
                        last_smm = smm

                # prefetch t+2; pin its vertical-max tree behind the end of
                # this tap loop so the list scheduler keeps it as filler for
                # the ln/exp combine latency instead of hoisting it early
                if t + 2 < NT:
                    load_tile(t + 2)
                    first = vmax_tree(t + 2)
                    add_dep_helper(first.ins, last_smm.ins, sync=True,
                                   reason="defer vmax filler")

                # ---- combine (f32): out = lam*(T/S) + (1-lam)*max ----
                lnS = pool.tile([128, R, W], F32, tag="lnS")
                nc.scalar.activation(lnS[:], S_ps[:], AF.Ln)
                rS = pool.tile([128, R, W], F32, tag="rS")
                rs_ins = nc.scalar.activation(rS[:], lnS[:], AF.Exp,
                                              scale=-1.0)
                MXt = MXs.pop(t)
                med = pool.tile([128, R, W], F32, tag="med")
                nc.vector.tensor_tensor(med[:], rS[:], T_ps[:], ALU.mult)
                nc.vector.tensor_tensor(med[:], med[:], MXt[:], ALU.subtract)
                nc.vector.tensor_scalar(med[:], med[:], LAM[t][:], None,
                                        ALU.mult)
                out_t = pool.tile([128, R, W], F32, tag="out")
                nc.vector.tensor_tensor(out_t[:], med[:], MXt[:], ALU.add)
                nc.sync.dma_start(
                    out=_dram_ap(y, t * SPT * HW,
                                 [[HW, SPT], [R * W, NSTRIP], [1, R * W]]),
                    in_=out_t[:],
                )
                X.pop(t)

                # ---- mean + horizontal max for tile t+1 (fills the gap
                # between ln(S_t) and the next tap loop) ----
                if t + 1 < NT:
                    first = hmax(t + 1)
                    add_dep_helper(first.ins, rs_ins.ins, sync=True,
                                   reason="defer hmax filler")
                    mean_pe(t + 1)
    _elide_covered_waits(nc)
    if split_waits:
        # Mechanical transform for walrus's 1-wait instruction formats;
        # skip under CoreSim (its race detector requires sem updates on
        # every instruction, which the injected bare NoOps lack).
        _split_excess_waits(nc)
    return nc


def _make_inputs(x, mix, beta_raw):
    """Host-side sharding. Returns (beta, in_maps)."""
    x = np.ascontiguousarray(x, dtype=np.float32)
    mix = np.asarray(mix, dtype=np.float32).reshape(C)
    beta_raw = float(np.asarray(beta_raw, dtype=np.float32))
    beta = float(5.0 + 45.0 / (1.0 + np.exp(-beta_raw)))
    lam_c = (1.0 / (1.0 + np.exp(-mix.astype(np.float64)))).astype(np.float32)

    xs_all = np.pad(x.reshape(B * C, H, W), ((0, 0), (2, 2), (2, 4)),
                    mode="reflect").astype(ml_dtypes.bfloat16)
    ident = np.eye(128, dtype=ml_dtypes.bfloat16)
    in_maps = []
    for core in range(NCORES):
        sl0 = core * SL
        shard = np.zeros(SL * SLICE_E + 8, dtype=ml_dtypes.bfloat16)
        shard[:SL * SLICE_E] = xs_all[sl0:sl0 + SL].reshape(-1)
        lam_t = np.empty((NT, 128, 1), dtype=np.float32)
        for t in range(NT):
            for p in range(128):
                g_slice = sl0 + t * SPT + p // NSTRIP
                lam_t[t, p, 0] = lam_c[g_slice % C]
        in_maps.append({"xs": shard, "lam": lam_t, "ident": ident})
    return beta, in_maps


def kernel(x, mix, beta_raw):
    beta, in_maps = _make_inputs(x, mix, beta_raw)
    nc = build_program(beta)
    res = run_bass_kernel_spmd(nc, in_maps, list(range(NCORES))).results
    out = np.concatenate([res[i]["y"].reshape(SL, H, W)
                          for i in range(NCORES)], axis=0)
    return np.ascontiguousarray(out.reshape(B, C, H, W))
